# revision 1
# baseline (speedup 1.0000x reference)
"""Trainium2 Bass kernel for CubicModel: out = feats(feats(x)@W0.T+b0)@W1.T+b1
where feats(z) = [z, triu(z_i z_j), z^3].

Strategy (8 NeuronCores, tensor-parallel over the 132352-dim feature axis):
  * The triu pair set is decomposed cyclically by difference class d: core c
    owns pairs (i, i+d mod 512) for i in a per-class 64-row window
    [64c + r_d, 64c + r_d + 64), r_d = (-d) mod 64.  The r_d shift makes both
    tensor_tensor operands start at the same legal partition base (SBUF engine
    APs must start at 0/32/64/96 and walrus requires equal input bases):
    in0 reads the shift-indexed array Y2 (parity-split: shift r lives at
    partition base 64*(r%2), free slot r//2), in1 reads a 64-strided frame
    block at the same base.  The device program is IDENTICAL on all cores --
    all per-core variation lives in the data (rotated xT blocks, Y2, permuted
    fp16 W^T, gather indices), as SPMD requires.
  * Features are generated directly in transposed [k, b] fp16 layout on
    DVE/GPSIMD tensor_tensor ops and fed to the PE as the moving operand;
    host-pre-transposed fp16 W^T tiles (4 k-tiles per DMA) are the stationary
    operand.  PSUM accumulates fp32 over 130 k-tiles.  K-tiles are processed
    in descending-d order so the ascending-r Y2 DMA chunks unblock the first
    tiles quickly.
  * Layer-0 partial hT: ReduceScatter + AllGather (fp32), per-core rotated
    re-load via indirect row gathers (bias-add + fp16 downcast on ACT); the
    layer-1 shift array Y2_h is built on-device with free-sliced identity
    matmuls (PE shift trick), interleaved ACT/DVE evacuation, in the same
    ascending-r order layer-1 generation consumes.  Output: ReduceScatter +
    per-core bias; host concatenates and transposes.
"""

import sys

sys.path.insert(0, "/opt/trn_rl_repo")

import numpy as np

N_CORES = 8
D = 512          # d_in == hidden
B = 512          # batch
H = 512          # hidden
DOUT = 256
ROT = D // N_CORES          # 64
KC = 16544                  # features per core (255*64 + 64 + 64 + 64 + 32)
KT = 130                    # k-tiles per core (last: 32 real rows + 96 pad)
QUAD_BASE = D
CUBIC_BASE = D + (D * D + D) // 2    # 131840
N_H_TILES = H // 128        # 4
N_O_TILES = DOUT // 128     # 2
WCHUNK = 4                  # k-tiles per weight DMA (130 = 32*4 + 2)

# processing order of logical k-tiles: descending d first (so Y2 shift columns
# are consumed in ascending-r order), then linear/cubic, then d256+pad
TILE_ORDER = list(range(127, -1, -1)) + [128, 129]

# every 4th position's generation ops go to GPSIMD instead of DVE
GPSIMD_TILE_STRIDE = 4


def _class_geom(d):
    """Shift geometry for difference class d in [1, 255]."""
    r = (-d) % 64            # in0 shift (Y2 column)
    e = r + d                # in1 start row in the core frame; 64*ceil(d/64)
    assert e % 64 == 0 and 64 <= e <= 256
    return r, e


def _core_schedule(c):
    """Reference feature-column index for each of the KC featsT rows
    (logical tile-index order, i.e. tile t occupies rows [128t, 128t+128))."""
    refk = np.empty(KC, dtype=np.int64)
    r = 0
    for d in range(1, 256):
        rd, _ = _class_geom(d)
        p = np.arange(64)
        i = (ROT * c + rd + p) % D
        j = (i + d) % D
        lo = np.minimum(i, j)
        hi = np.maximum(i, j)
        refk[r:r + 64] = QUAD_BASE + lo * D - lo * (lo - 1) // 2 + (hi - lo)
        r += 64
    p = np.arange(64)
    i = ROT * c + p
    refk[r:r + 64] = QUAD_BASE + i * D - i * (i - 1) // 2      # loops x_i^2
    r += 64
    refk[r:r + 64] = i                                          # linear
    r += 64
    refk[r:r + 64] = CUBIC_BASE + i                             # cubic
    r += 64
    k = np.arange(32)
    a = 32 * c + k                                              # d = 256 class
    refk[r:r + 32] = QUAD_BASE + a * D - a * (a - 1) // 2 + 256
    r += 32
    assert r == KC
    return refk


def _pack_y2(rows16T):
    """Parity-split shift array: Y2[64*(r%2)+p, r//2, :] = src[r + p]."""
    y2 = np.zeros((128, 32, B), dtype=np.float16)
    for r in range(64):
        y2[64 * (r % 2):64 * (r % 2) + 64, r // 2, :] = rows16T[r:r + 64]
    return y2


def _prep_core_inputs(c, x16T, W0T, W1T, b0, b1):
    refk = _core_schedule(c)
    kc_pad = KT * 128

    w0t = np.zeros((kc_pad, H), dtype=np.float16)
    w0t[:KC] = W0T[refk]
    w1t = np.zeros((kc_pad, DOUT), dtype=np.float16)
    w1t[:KC] = W1T[refk]
    # reorder k-tiles into processing order
    w0t = np.ascontiguousarray(w0t.reshape(KT, 128, H)[TILE_ORDER])
    w1t = np.ascontiguousarray(w1t.reshape(KT, 128, DOUT)[TILE_ORDER])

    # frame blocks q in [0,5): 64-strided windows rows (64c + 64q + p) % 512;
    # q=5: d256 in1 ; q=6: d256 in0
    xtw = np.zeros((128, 7, B), dtype=np.float16)
    hidx = np.zeros((128, 7), dtype=np.int32)
    p = np.arange(128)
    for q in range(5):
        rows = (ROT * c + 64 * q + p) % D
        xtw[:, q, :] = x16T[rows]
        hidx[:, q] = rows
    rows5 = (32 * c + 256 + np.arange(32)) % D
    xtw[:32, 5, :] = x16T[rows5]
    hidx[:32, 5] = rows5
    rows6 = 32 * c + np.arange(32)
    xtw[:32, 6, :] = x16T[rows6]
    hidx[:32, 6] = rows6

    # shift rows [0, 128) of the core frame
    frame = x16T[(ROT * c + np.arange(128)) % D]
    y2 = _pack_y2(frame)

    ident = np.eye(128, dtype=np.float16)

    b0p = b0[64 * c:64 * c + 64].astype(np.float32).reshape(64, 1)
    b1w = b1[32 * c:32 * c + 32].astype(np.float32).reshape(32, 1)

    return {
        "w0t": w0t,
        "w1t": w1t,
        "xtw": xtw,
        "y2": y2,
        "ident": ident,
        "hidx": hidx,
        "b0p": b0p,
        "b1w": b1w,
    }


def _emit_gen_ops(nc, s, t, ft, src, ysrc, sq64, on_gpsimd):
    """Feature-generation ops for logical k-tile t (position s) into ft.

    src: [128, 7, B] fp16 frame blocks (+d256 operands); ysrc: [128, 32, B]
    parity-split shift array; sq64: [64, B] staging for cubes.
    """
    eng = nc.gpsimd if on_gpsimd else nc.vector
    if t < 128:
        for u in (0, 1):
            d = 2 * t + u + 1
            if d <= 255:
                r, e = _class_geom(d)
                par = r % 2
                base = 64 * par
                q = e // 64 - par           # frame block giving rows [e, e+64) at `base`
                eng.tensor_mul(ft[64 * u:64 * u + 64, :],
                               ysrc[base:base + 64, r // 2, :],
                               src[base:base + 64, q, :])
            else:  # t == 127, u == 1: loops x_i^2 (r=0 column of Y2)
                eng.tensor_mul(ft[64:128, :], ysrc[0:64, 0, :], ysrc[0:64, 0, :])
    elif t == 128:
        eng.tensor_copy(ft[0:64, :], ysrc[0:64, 0, :])           # linear
        eng.tensor_mul(sq64[0:64, :], ysrc[0:64, 0, :], ysrc[0:64, 0, :])
        eng.tensor_mul(ft[64:128, :], sq64[0:64, :], ysrc[0:64, 0, :])  # cubic
    else:  # t == 129: d=256 class + zero padding
        eng.tensor_mul(ft[0:32, :], src[0:32, 6, :], src[0:32, 5, :])
        eng.memset(ft[32:64, :], 0)
        eng.memset(ft[64:128, :], 0)


def _build_program(repeat=1):
    import concourse.mybir as mybir
    import concourse.tile as tile
    from concourse import bacc
    from concourse.bass import IndirectOffsetOnAxis

    fp16 = mybir.dt.float16
    f32 = mybir.dt.float32
    Copy = mybir.ActivationFunctionType.Copy
    Ident = mybir.ActivationFunctionType.Identity
    CORE_IDS = list(range(N_CORES))

    import contextlib

    nc = bacc.Bacc(None, target_bir_lowering=False, debug=False)
    with tile.TileContext(nc) as tc:
        with tc.tile_pool(name="dram", bufs=1, space="DRAM") as dram, \
             tc.tile_pool(name="const", bufs=1) as const, \
             tc.tile_pool(name="wpool", bufs=6) as wpool, \
             tc.tile_pool(name="fpool", bufs=8) as fpool, \
             tc.tile_pool(name="spool", bufs=4) as spool, \
             tc.tile_pool(name="ps", bufs=1, space="PSUM") as ps:
            # ---- DRAM I/O ----
            w0t = dram.tile([KT, 128, H], fp16, kind="ExternalInput", name="w0t", uniquify=False)
            w1t = dram.tile([KT, 128, DOUT], fp16, kind="ExternalInput", name="w1t", uniquify=False)
            xtw = dram.tile([128, 7, B], fp16, kind="ExternalInput", name="xtw", uniquify=False)
            y2d = dram.tile([128, 32, B], fp16, kind="ExternalInput", name="y2", uniquify=False)
            identd = dram.tile([128, 128], fp16, kind="ExternalInput", name="ident", uniquify=False)
            hidx = dram.tile([128, 7], mybir.dt.int32, kind="ExternalInput", name="hidx", uniquify=False)
            b0p = dram.tile([64, 1], f32, kind="ExternalInput", name="b0p", uniquify=False)
            b1w = dram.tile([32, 1], f32, kind="ExternalInput", name="b1w", uniquify=False)
            outp = dram.tile([DOUT // N_CORES, B], f32, kind="ExternalOutput", name="outp", uniquify=False)
            cc0_in = dram.tile([H, B], f32, name="cc0_in", uniquify=False)
            rs0_out = dram.tile([H // N_CORES, B], f32, name="rs0_out", uniquify=False)
            cc0h_in = dram.tile([H // N_CORES, B], fp16, name="cc0h_in", uniquify=False)
            cc0_out = dram.tile([H, B], fp16, name="cc0_out", uniquify=False, addr_space="Shared")
            cc1_in = dram.tile([DOUT, B], f32, name="cc1_in", uniquify=False)
            rs_out = dram.tile([DOUT // N_CORES, B], f32, name="rs_out", uniquify=False)

            rep_cm = tc.For_i(0, repeat, 1) if repeat > 1 else contextlib.nullcontext()
            with rep_cm:
                # ---- layer-0 constants (Y2 in 4 ascending-r chunks) ----
                xtw_sb = const.tile([128, 7, B], fp16)
                nc.sync.dma_start(out=xtw_sb, in_=xtw[:])
                y2_sb = const.tile([128, 32, B], fp16, tag="yshift")
                nc.sync.dma_start(out=y2_sb[:, 0:8, :], in_=y2d[:, 0:8, :])

                _y2state = {"next": 1}

                def _rest_of_y2():
                    hi = 2 if _y2state["next"] == 1 else 4
                    for ch in range(_y2state["next"], hi):
                        nc.sync.dma_start(out=y2_sb[:, 8 * ch:8 * ch + 8, :],
                                          in_=y2d[:, 8 * ch:8 * ch + 8, :])
                    _y2state["next"] = hi
                sq64 = const.tile([64, B], fp16)
                sq64h = const.tile([64, B], fp16)

                def run_layer(src_sb, ysrc_sb, w_dram, n_m_tiles, cc_in_dram, sq,
                              gp_pred, after_first_w=None):
                    psums = [ps.tile([128, B], f32, tag=f"bank{h}",
                                     name=f"psum{n_m_tiles}_{h}")
                             for h in range(n_m_tiles)]
                    w_sbs = {}
                    for s0 in range(0, KT, WCHUNK):
                        nw = min(WCHUNK, KT - s0)
                        w_sb = wpool.tile([128, WCHUNK, n_m_tiles * 128], fp16,
                                          tag=f"w{n_m_tiles}", name=f"wsb{n_m_tiles}_{s0}")
                        nc.sync.dma_start(
                            out=w_sb[:, 0:nw, :],
                            in_=w_dram[s0:s0 + nw].rearrange("k p h -> p k h"))
                        w_sbs[s0] = w_sb
                        if s0 in (0, WCHUNK) and after_first_w is not None:
                            after_first_w()
                    for s in range(KT):
                        t = TILE_ORDER[s]
                        ft = fpool.tile([128, B], fp16, tag="ft")
                        _emit_gen_ops(nc, s, t, ft, src_sb, ysrc_sb, sq, gp_pred(s))
                        w_sb = w_sbs[(s // WCHUNK) * WCHUNK]
                        kk = s % WCHUNK
                        for h in range(n_m_tiles):
                            nc.tensor.matmul(psums[h],
                                             w_sb[:, kk, 128 * h:128 * h + 128], ft,
                                             start=(s == 0), stop=(s == KT - 1))
                    for h in range(n_m_tiles):
                        stage = spool.tile([128, B], f32, tag="evac")
                        nc.scalar.activation(stage, psums[h], Copy)
                        nc.sync.dma_start(out=cc_in_dram[128 * h:128 * h + 128, :], in_=stage)

                # ================= layer 0 =================
                run_layer(xtw_sb, y2_sb, w0t, N_H_TILES, cc0_in, sq64,
                          gp_pred=lambda s: s % 8 == 7, after_first_w=_rest_of_y2)

                # ---- layer-1 constants (loaded late, off the critical path) ----
                ident_sb = const.tile([128, 128], fp16)
                nc.sync.dma_start(out=ident_sb, in_=identd[:])
                hidx_sb = const.tile([128, 7], mybir.dt.int32)
                nc.sync.dma_start(out=hidx_sb, in_=hidx[:])
                b0p_sb = const.tile([64, 1], f32)
                nc.sync.dma_start(out=b0p_sb, in_=b0p[:])
                b1w_sb = const.tile([32, 1], f32)
                nc.sync.dma_start(out=b1w_sb, in_=b1w[:])

                nc.gpsimd.collective_compute(
                    "ReduceScatter", mybir.AluOpType.add,
                    replica_groups=[CORE_IDS], ins=[cc0_in[:]], outs=[rs0_out[:]],
                )
                # local bias-add + fp16 downcast on the owned 64-row piece
                rsp_sb = spool.tile([64, B], f32, tag="rsp")
                nc.sync.dma_start(out=rsp_sb, in_=rs0_out[:])
                rsp16_sb = spool.tile([64, B], fp16, tag="rsp16")
                nc.scalar.activation(rsp16_sb, rsp_sb, Ident, bias=b0p_sb[:, 0:1])
                nc.sync.dma_start(out=cc0h_in[:], in_=rsp16_sb)
                nc.gpsimd.collective_compute(
                    "AllGather", mybir.AluOpType.bypass,
                    replica_groups=[CORE_IDS], ins=[cc0h_in[:]], outs=[cc0_out[:]],
                )
                # gather rotated h rows (bias already applied, fp16)
                htw_sb = const.tile([128, 7, B], fp16)
                for q in range(7):
                    nc.gpsimd.indirect_dma_start(
                        out=htw_sb[:, q, :], out_offset=None, in_=cc0_out[:],
                        in_offset=IndirectOffsetOnAxis(ap=hidx_sb[:, q:q + 1], axis=0),
                    )

                # build Y2_h on-device in ascending-r order via identity-shift
                # matmuls: psum = ident[:, r:r+64].T @ htw block0 = rows [r, r+64)
                y2h_sb = const.tile([128, 32, B], fp16, tag="yshift2")
                for r in range(64):
                    base = 64 * (r % 2)
                    psy = ps.tile([128, B], f32, tag=f"ybank{r % 2}", name=f"psy{r}", bufs=2)
                    nc.tensor.matmul(psy[base:base + 64, :],
                                     ident_sb[:, r:r + 64], htw_sb[:, 0, :],
                                     start=True, stop=True,
                                     tile_position=(0, base))
                    nc.scalar.activation(y2h_sb[base:base + 64, r // 2, :],
                                         psy[base:base + 64, :], Copy)

                # ================= layer 1 =================
                run_layer(htw_sb, y2h_sb, w1t, N_O_TILES, cc1_in, sq64h,
                          gp_pred=lambda s: s % 7 in (1, 3, 5))
                nc.gpsimd.collective_compute(
                    "ReduceScatter", mybir.AluOpType.add,
                    replica_groups=[CORE_IDS], ins=[cc1_in[:]], outs=[rs_out[:]],
                )
                rs_sb = spool.tile([DOUT // N_CORES, B], f32, tag="rs")
                nc.sync.dma_start(out=rs_sb, in_=rs_out[:])
                out_sb = spool.tile([DOUT // N_CORES, B], f32, tag="outsb")
                nc.scalar.activation(out_sb, rs_sb, Ident, bias=b1w_sb[:, 0:1])
                nc.sync.dma_start(out=outp[:], in_=out_sb)
    nc.compile()
    return nc


_NC_CACHE = None


def build_in_maps(x, W0, b0, W1, b1):
    x16T = np.ascontiguousarray(x.T).astype(np.float16)          # [D, B]
    W0T = np.ascontiguousarray(W0.T).astype(np.float16)          # [K, H]
    W1T = np.ascontiguousarray(W1.T).astype(np.float16)          # [K, DOUT]
    return [_prep_core_inputs(c, x16T, W0T, W1T, b0, b1) for c in range(N_CORES)]


def kernel(x, W0, b0, W1, b1):
    global _NC_CACHE
    from concourse.bass_utils import run_bass_kernel_spmd

    in_maps = build_in_maps(x, W0, b0, W1, b1)
    if _NC_CACHE is None:
        _NC_CACHE = _build_program()
    res = run_bass_kernel_spmd(_NC_CACHE, in_maps, list(range(N_CORES)))
    outT = np.concatenate([res.results[c]["outp"] for c in range(N_CORES)], axis=0)
    return np.ascontiguousarray(outT.T.astype(np.float32))



# revision 5
# speedup vs baseline: 1.1718x; 1.1718x over previous
"""Trainium2 Bass kernel for CubicModel: out = feats(feats(x)@W0.T+b0)@W1.T+b1
where feats(z) = [z, triu(z_i z_j), z^3].

Strategy (8 NeuronCores, tensor-parallel over the 132352-dim feature axis):
  * Fused feature generation: each 128-row k-tile is produced by ONE full-width
    DVE tensor_tensor op.  For a quad tile t (classes d0=2t+1, d1=2t+2) the two
    64-row halves share the same in1 window (rows [e, e+64) of the core frame,
    e = 64*(t//32+1), replicated across both partition halves in `xrep`), and
    the in0 shifts r1, r0=r1+1 sit exactly in one column of the parity-packed
    shift array Y2.  Tile rows are ordered [class d1; class d0] and the W rows
    are permuted to match (all per-core variation lives in the data).
  * The two batch halves are pipelined with a lag of LAG k-tiles: the PE stream
    interleaves half-A tile s with half-B tile s-LAG, so W0 is streamed ONCE
    (ring of LAG/WCHUNK+2 buffers) while half-A's ReduceScatter + AllGather +
    operand rebuild hides under half-B compute.  W1 is small and is streamed
    twice (once per half) through a short ring.
  * Layer-0 partial h: fp16 ReduceScatter -> local bias+fp16 -> AllGather ->
    4 indirect row-gathers build the rotated h-frame in SBUF, the frame is
    written to DRAM in frame order, and plain strided-window DMAs build the
    layer-1 shift array Y2_h and window operands (no PE identity trick).
  * Output: NO final collective.  Each core writes its full fp32 partial
    [256, B]; the host sums the 8 partials and adds b1.
  * Engine roles: SP DGE streams weights + Y2_x only; Pool DGE issues all
    latency-critical glue DMAs, indirect gathers and collectives; ACT does
    PSUM evacuation + bias; DVE does feature generation only.
"""

import sys

sys.path.insert(0, "/opt/trn_rl_repo")

import numpy as np

N_CORES = 8
D = 512          # d_in == hidden
B = 512          # batch
H = 512          # hidden
DOUT = 256
ROT = D // N_CORES          # 64
KT = 130                    # k-tiles per core
QUAD_BASE = D
CUBIC_BASE = D + (D * D + D) // 2    # 131840
N_H_TILES = H // 128        # 4
N_O_TILES = DOUT // 128     # 2
WCHUNK = 4                  # k-tiles per weight DMA
LAG = 80                    # half-B k-tile lag behind half-A
BH = B // 2                 # 256 batch cols per half

# ---------------------------------------------------------------------------
# Tile schedule: position s -> logical tile.
#  ('SQ',): rows 0-63 squares x_i^2, rows 64-127 cubes x_i^3   (i=64c+p)
#  ('D256',): rows 0-31 pairs (a, a+256) a=32c+k, rows 32-127 zero pad
#  ('L255',): rows 0-63 linear x_i (i=64c+p), rows 64-127 class d=255
#  ('Q', t): rows 0-63 class d1=2t+2, rows 64-127 class d0=2t+1
# Quad tiles ordered by ascending Y2 column m = 31 - t%32 so the layer-1 shift
# array can be consumed as it is built.
# ---------------------------------------------------------------------------
PROC = [("SQ",), ("D256",), ("L255",)]
for _col in range(32):
    for _t in (31 - _col, 63 - _col, 95 - _col, 127 - _col):
        if _t <= 126:
            PROC.append(("Q", _t))
assert len(PROC) == KT


def _triu_idx(lo, hi):
    return QUAD_BASE + lo * D - lo * (lo - 1) // 2 + (hi - lo)


def _pair_fk(i, d):
    """Feature index for pair {i, (i+d) mod D} (arrays ok)."""
    j = (i + d) % D
    lo = np.minimum(i, j)
    hi = np.maximum(i, j)
    return _triu_idx(lo, hi)


def _core_refk(c):
    """W-row (feature) index for each tile row, in PROC order. -1 = zero pad."""
    refk = np.full((KT, 128), -1, dtype=np.int64)
    p = np.arange(64)
    base = ROT * c
    for s, desc in enumerate(PROC):
        if desc[0] == "SQ":
            i = (base + p) % D
            refk[s, 0:64] = _triu_idx(i, i)
            refk[s, 64:128] = CUBIC_BASE + i
        elif desc[0] == "D256":
            a = 32 * c + np.arange(32)
            refk[s, 0:32] = _triu_idx(a, a + 256)
        elif desc[0] == "L255":
            i = (base + p) % D
            refk[s, 0:64] = i
            refk[s, 64:128] = _pair_fk((base + 1 + p) % D, 255)
        else:
            t = desc[1]
            d1, d0 = 2 * t + 2, 2 * t + 1
            r1 = (-d1) % 64
            r0 = r1 + 1
            refk[s, 0:64] = _pair_fk((base + r1 + p) % D, d1)
            refk[s, 64:128] = _pair_fk((base + r0 + p) % D, d0)
    return refk


def _pack_y2(rows16T, b):
    """Parity-split shift array: Y2[64*(r%2)+p, r//2, :] = src[r + p]."""
    y2 = np.zeros((128, 32, b), dtype=np.float16)
    for r in range(64):
        y2[64 * (r % 2):64 * (r % 2) + 64, r // 2, :] = rows16T[r:r + 64]
    return y2


def _prep_core_inputs(c, x16T, W0T, W1T, b0):
    refk = _core_refk(c)

    w0t = np.zeros((KT, 128, H), dtype=np.float16)
    w1t = np.zeros((KT, 128, DOUT), dtype=np.float16)
    m = refk >= 0
    w0t[m] = W0T[refk[m]]
    w1t[m] = W1T[refk[m]]

    frame = x16T[(ROT * c + np.arange(320)) % D]      # rotated 320-row frame
    y2x = _pack_y2(frame, B)

    p = np.arange(64)
    xrep = np.zeros((128, 5, B), dtype=np.float16)
    for m5 in range(5):
        xrep[0:64, m5] = frame[64 * m5 + p]
        xrep[64:128, m5] = frame[64 * m5 + p]
    xmix = np.zeros((128, 2, B), dtype=np.float16)
    xmix[0:64, :, :] = 1.0
    xmix[64:128, 0] = frame[p]
    xmix[64:128, 1] = frame[256 + p]
    xd256 = np.zeros((128, 2, B), dtype=np.float16)
    k32 = np.arange(32)
    xd256[0:32, 0] = x16T[(32 * c + k32) % D]
    xd256[0:32, 1] = x16T[(32 * c + 256 + k32) % D]

    # indirect-gather row indices for the layer-1 h-frame rebuild
    pp = np.arange(128)
    hfridx = np.zeros((128, 4), dtype=np.int32)
    for q in range(3):
        hfridx[:, q] = (ROT * c + 128 * q + pp) % D
    hfridx[0:32, 3] = (32 * c + k32) % D
    hfridx[32:64, 3] = (32 * c + 256 + k32) % D

    b0p = b0[ROT * c:ROT * c + 64].astype(np.float32).reshape(64, 1)

    return {
        "w0t": w0t,
        "w1t": w1t,
        "y2x": y2x,
        "xrep": xrep,
        "xmix": xmix,
        "xd256": xd256,
        "hfridx": hfridx,
        "b0p": b0p,
    }


def _emit_gen(nc, s, ft, y2, xrep, xmix, xd256, sq, lo, hi):
    """One fused gen op for tile s into ft (batch-half cols [lo, hi))."""
    desc = PROC[s]
    v = nc.vector
    if desc[0] == "SQ":
        v.tensor_mul(sq[:, lo:hi], xrep[:, 0, lo:hi], xrep[:, 0, lo:hi])
        v.tensor_mul(ft[:, :], sq[:, lo:hi], xmix[:, 0, lo:hi])
    elif desc[0] == "D256":
        v.tensor_mul(ft[:, :], xd256[:, 0, lo:hi], xd256[:, 1, lo:hi])
    elif desc[0] == "L255":
        v.tensor_mul(ft[:, :], y2[:, 0, lo:hi], xmix[:, 1, lo:hi])
    else:
        t = desc[1]
        v.tensor_mul(ft[:, :], y2[:, 31 - (t % 32), lo:hi],
                     xrep[:, t // 32 + 1, lo:hi])


def _build_program():
    import concourse.mybir as mybir
    import concourse.tile as tile
    from concourse import bacc
    from concourse.bass import AP, IndirectOffsetOnAxis

    fp16 = mybir.dt.float16
    f32 = mybir.dt.float32
    i32 = mybir.dt.int32
    Copy = mybir.ActivationFunctionType.Copy
    Ident = mybir.ActivationFunctionType.Identity
    CORE_IDS = list(range(N_CORES))

    nc = bacc.Bacc(None, target_bir_lowering=False, debug=False)
    with tile.TileContext(nc) as tc:
        with tc.tile_pool(name="dram", bufs=1, space="DRAM") as dram, \
             tc.tile_pool(name="const", bufs=1) as const, \
             tc.tile_pool(name="w0pool", bufs=LAG // WCHUNK + 2) as w0pool, \
             tc.tile_pool(name="w1pool", bufs=6) as w1pool, \
             tc.tile_pool(name="fpool", bufs=8) as fpool, \
             tc.tile_pool(name="spool", bufs=2) as spool, \
             tc.tile_pool(name="hpool", bufs=2) as hpool, \
             tc.tile_pool(name="ps", bufs=1, space="PSUM") as ps:
            # ---- DRAM I/O ----
            w0t = dram.tile([KT, 128, H], fp16, kind="ExternalInput", name="w0t", uniquify=False)
            w1t = dram.tile([KT, 128, DOUT], fp16, kind="ExternalInput", name="w1t", uniquify=False)
            y2xd = dram.tile([128, 32, B], fp16, kind="ExternalInput", name="y2x", uniquify=False)
            xrepd = dram.tile([128, 5, B], fp16, kind="ExternalInput", name="xrep", uniquify=False)
            xmixd = dram.tile([128, 2, B], fp16, kind="ExternalInput", name="xmix", uniquify=False)
            xd256d = dram.tile([128, 2, B], fp16, kind="ExternalInput", name="xd256", uniquify=False)
            hfridxd = dram.tile([128, 4], i32, kind="ExternalInput", name="hfridx", uniquify=False)
            b0pd = dram.tile([64, 1], f32, kind="ExternalInput", name="b0p", uniquify=False)
            outp = dram.tile([DOUT, B], f32, kind="ExternalOutput", name="outp", uniquify=False)

            halves = []
            for hn, lo in (("A", 0), ("B", BH)):
                halves.append({
                    "lo": lo, "hi": lo + BH, "tag": hn,
                    "cc": dram.tile([H, BH], fp16, name=f"cc{hn}", uniquify=False),
                    "rs": dram.tile([ROT, BH], fp16, name=f"rs{hn}", uniquify=False),
                    "hpc": dram.tile([ROT, BH], fp16, name=f"hpc{hn}", uniquify=False),
                    "ag": dram.tile([H, BH], fp16, name=f"ag{hn}", uniquify=False,
                                    addr_space="Shared"),
                    "hfr": dram.tile([448, BH], fp16, name=f"hfr{hn}", uniquify=False),
                })

            # ---- small constants (Pool DGE: fast issue) ----
            xrep_sb = const.tile([128, 5, B], fp16)
            nc.gpsimd.dma_start(out=xrep_sb, in_=xrepd[:])
            xmix_sb = const.tile([128, 2, B], fp16)
            nc.gpsimd.dma_start(out=xmix_sb, in_=xmixd[:])
            xd256_sb = const.tile([128, 2, B], fp16)
            nc.gpsimd.dma_start(out=xd256_sb, in_=xd256d[:])
            hfridx_sb = const.tile([128, 4], i32)
            nc.gpsimd.dma_start(out=hfridx_sb, in_=hfridxd[:])
            b0p_sb = const.tile([64, 1], f32)
            nc.gpsimd.dma_start(out=b0p_sb, in_=b0pd[:])

            # Y2_x in 4 chunks on SP (first small chunk unblocks generation)
            y2x_sb = const.tile([128, 32, B], fp16, tag="y2x")
            for (c0, c1) in [(0, 4), (4, 12), (12, 22), (22, 32)]:
                nc.sync.dma_start(out=y2x_sb[:, c0:c1, :], in_=y2xd[:, c0:c1, :])
            sqA = const.tile([128, B], fp16, tag="sq")

            for hf in halves:
                hf["ps0"] = [ps.tile([128, BH], f32, tag=f"p{hf['tag']}{h}",
                                     name=f"ps0{hf['tag']}{h}")
                             for h in range(N_H_TILES)]

            # ---- weight streaming ----
            def load_w(dram_t, sbs, pool, s0, nmt, tagn, eng):
                nw = min(WCHUNK, KT - s0)
                w_sb = pool.tile([128, WCHUNK, nmt * 128], fp16, tag=tagn)
                eng.dma_start(out=w_sb[:, 0:nw, :],
                              in_=dram_t[s0:s0 + nw].rearrange("k p h -> p k h"))
                sbs[s0] = w_sb

            w0_sbs = {}
            for s0 in range(0, KT, WCHUNK):
                load_w(w0t, w0_sbs, w0pool, s0, N_H_TILES, "w0", nc.sync)

            def emit_l0(hf, s):
                ft = fpool.tile([128, BH], fp16, tag="ft")
                _emit_gen(nc, s, ft, y2x_sb, xrep_sb, xmix_sb, xd256_sb,
                          sqA, hf["lo"], hf["hi"])
                w_sb = w0_sbs[(s // WCHUNK) * WCHUNK]
                kk = s % WCHUNK
                for h in range(N_H_TILES):
                    nc.tensor.matmul(hf["ps0"][h],
                                     w_sb[:, kk, 128 * h:128 * h + 128], ft,
                                     start=(s == 0), stop=(s == KT - 1))

            def emit_exchange(hf):
                """Evac psums -> RS -> bias fp16 -> AG -> rebuild h operands."""
                stage = spool.tile([128, N_H_TILES, BH], fp16, tag="evac")
                for h in range(N_H_TILES):
                    nc.scalar.activation(stage[:, h, :], hf["ps0"][h], Copy)
                    nc.gpsimd.dma_start(out=hf["cc"][128 * h:128 * h + 128, :],
                                        in_=stage[:, h, :])
                nc.gpsimd.collective_compute(
                    "ReduceScatter", mybir.AluOpType.add,
                    replica_groups=[CORE_IDS], ins=[hf["cc"][:]], outs=[hf["rs"][:]],
                )
                rsp = spool.tile([64, BH], fp16, tag="rsp")
                nc.gpsimd.dma_start(out=rsp, in_=hf["rs"][:])
                rsp16 = spool.tile([64, BH], fp16, tag="rsp16")
                nc.scalar.activation(rsp16, rsp, Ident, bias=b0p_sb[:, 0:1])
                nc.gpsimd.dma_start(out=hf["hpc"][:], in_=rsp16)
                nc.gpsimd.collective_compute(
                    "AllGather", mybir.AluOpType.bypass,
                    replica_groups=[CORE_IDS], ins=[hf["hpc"][:]], outs=[hf["ag"][:]],
                )
                # rotated h-frame rebuild: 4 indirect gathers -> SBUF -> DRAM
                hfs = hpool.tile([128, 4, BH], fp16, tag="hfs")
                for q in range(3):
                    nc.gpsimd.indirect_dma_start(
                        out=hfs[:, q, :], out_offset=None, in_=hf["ag"][:],
                        in_offset=IndirectOffsetOnAxis(ap=hfridx_sb[:, q:q + 1], axis=0),
                    )
                nc.gpsimd.indirect_dma_start(
                    out=hfs[0:64, 3, :], out_offset=None, in_=hf["ag"][:],
                    in_offset=IndirectOffsetOnAxis(ap=hfridx_sb[0:64, 3:4], axis=0),
                )
                hfrd = hf["hfr"]
                for q in range(3):
                    nc.gpsimd.dma_start(out=hfrd[128 * q:128 * q + 128, :],
                                        in_=hfs[:, q, :])
                nc.gpsimd.dma_start(out=hfrd[384:448, :], in_=hfs[0:64, 3, :])

                def win(row0, pn, sn, sstride_rows):
                    base = hfrd[row0:row0 + 1, :]
                    return AP(base.tensor, base.offset,
                              [[BH, pn], [sstride_rows * BH, sn], [1, BH]])

                # y2h: parity-packed shift array, first 8 columns prioritized
                y2h = hpool.tile([128, 32, BH], fp16, tag="y2h")
                for (s0, s1) in [(0, 8), (8, 32)]:
                    nc.gpsimd.dma_start(out=y2h[0:64, s0:s1, :],
                                        in_=win(2 * s0, 64, s1 - s0, 2))
                    nc.gpsimd.dma_start(out=y2h[64:128, s0:s1, :],
                                        in_=win(2 * s0 + 1, 64, s1 - s0, 2))
                # hrep: rows [64m, 64m+64) replicated in both partition halves
                hrep = hpool.tile([128, 5, BH], fp16, tag="hrep")
                nc.gpsimd.dma_start(out=hrep[0:64, :, :], in_=win(0, 64, 5, 64))
                nc.gpsimd.dma_start(out=hrep[64:128, :, :], in_=win(0, 64, 5, 64))
                # hmix: [ones; rows 0:64], [ones; rows 256:320]
                hmix = hpool.tile([128, 2, BH], fp16, tag="hmix")
                nc.vector.memset(hmix[0:64, :, :], 1.0)
                nc.gpsimd.dma_start(out=hmix[64:128, :, :], in_=win(0, 64, 2, 256))
                # hd256: in0 = [d256a; zeros], in1 = [d256b; zeros]
                hd256 = hpool.tile([128, 2, BH], fp16, tag="hd256")
                nc.vector.memset(hd256[32:64, :, :], 0.0)
                nc.vector.memset(hd256[64:128, :, :], 0.0)
                nc.gpsimd.dma_start(out=hd256[0:32, :, :], in_=win(384, 32, 2, 32))
                sqh = hpool.tile([128, BH], fp16, tag="sqh")
                hf["h_ops"] = (y2h, hrep, hmix, hd256, sqh)
                hf["ps1"] = [ps.tile([128, BH], f32, tag=f"p{hf['tag']}{h}",
                                     name=f"ps1{hf['tag']}{h}")
                             for h in range(N_O_TILES)]

            def emit_l1(hf, s, w_sbs):
                y2h, hrep, hmix, hd256, sqh = hf["h_ops"]
                ft = fpool.tile([128, BH], fp16, tag="ft")
                _emit_gen(nc, s, ft, y2h, hrep, hmix, hd256, sqh, 0, BH)
                w_sb = w_sbs[(s // WCHUNK) * WCHUNK]
                kk = s % WCHUNK
                for h in range(N_O_TILES):
                    nc.tensor.matmul(hf["ps1"][h],
                                     w_sb[:, kk, 128 * h:128 * h + 128], ft,
                                     start=(s == 0), stop=(s == KT - 1))

            def emit_out(hf):
                lo, hi = hf["lo"], hf["hi"]
                for h in range(N_O_TILES):
                    so = spool.tile([128, BH], f32, tag="oevac")
                    nc.scalar.activation(so, hf["ps1"][h], Copy)
                    nc.gpsimd.dma_start(out=outp[128 * h:128 * h + 128, lo:hi],
                                        in_=so)

            # ================= layer 0: interleaved halves =================
            A, Bhf = halves
            for s in range(KT + LAG):
                if s < KT:
                    emit_l0(A, s)
                if s == KT - 1:
                    emit_exchange(A)
                if s >= LAG:
                    emit_l0(Bhf, s - LAG)
            emit_exchange(Bhf)

            # ================= layer 1: sequential halves =================
            w1a_sbs = {}
            w1b_sbs = {}
            for s0 in range(0, KT, WCHUNK):
                load_w(w1t, w1a_sbs, w1pool, s0, N_O_TILES, "w1", nc.sync)
            for s0 in range(0, KT, WCHUNK):
                load_w(w1t, w1b_sbs, w1pool, s0, N_O_TILES, "w1", nc.sync)
            for s in range(KT):
                emit_l1(A, s, w1a_sbs)
            emit_out(A)
            for s in range(KT):
                emit_l1(Bhf, s, w1b_sbs)
            emit_out(Bhf)
    nc.compile()
    return nc


_NC_CACHE = None


def build_in_maps(x, W0, b0, W1, b1):
    x16T = np.ascontiguousarray(x.T).astype(np.float16)          # [D, B]
    W0T = np.ascontiguousarray(W0.T).astype(np.float16)          # [K, H]
    W1T = np.ascontiguousarray(W1.T).astype(np.float16)          # [K, DOUT]
    return [_prep_core_inputs(c, x16T, W0T, W1T, b0) for c in range(N_CORES)]


def kernel(x, W0, b0, W1, b1):
    global _NC_CACHE
    from concourse.bass_utils import run_bass_kernel_spmd

    in_maps = build_in_maps(x, W0, b0, W1, b1)
    if _NC_CACHE is None:
        _NC_CACHE = _build_program()
    res = run_bass_kernel_spmd(_NC_CACHE, in_maps, list(range(N_CORES)))
    acc = np.zeros((DOUT, B), dtype=np.float32)
    for c in range(N_CORES):
        acc += res.results[c]["outp"]
    acc += b1.astype(np.float32).reshape(DOUT, 1)
    return np.ascontiguousarray(acc.T)


# revision 8
# speedup vs baseline: 1.2698x; 1.0836x over previous
"""Trainium2 Bass kernel for CubicModel: out = feats(feats(x)@W0.T+b0)@W1.T+b1
where feats(z) = [z, triu(z_i z_j), z^3].

Strategy (8 NeuronCores, tensor-parallel over the 132352-dim feature axis):
  * Fused feature generation: each 128-row k-tile is produced by ONE full-width
    DVE tensor_tensor op.  For a quad tile t (classes d0=2t+1, d1=2t+2) the two
    64-row halves share the same in1 window (rows [e, e+64) of the core frame,
    e = 64*(t//32+1), replicated across both partition halves in `xrep`), and
    the in0 shifts r1, r0=r1+1 sit exactly in one column of the parity-packed
    shift array Y2.  Tile rows are ordered [class d1; class d0] and the W rows
    are permuted to match (all per-core variation lives in the data).
  * The two batch halves are pipelined with a lag of LAG k-tiles: the PE stream
    interleaves half-A tile s with half-B tile s-LAG, so W0 is streamed ONCE
    (ring of LAG/WCHUNK+2 buffers) while half-A's ReduceScatter + AllGather +
    operand rebuild hides under half-B compute.  W1 is small and is streamed
    twice (once per half) through a short ring.
  * Layer-0 partial h: fp16 ReduceScatter -> local bias+fp16 -> AllGather ->
    4 indirect row-gathers build the rotated h-frame in SBUF, the frame is
    written to DRAM in frame order, and plain strided-window DMAs build the
    layer-1 shift array Y2_h and window operands (no PE identity trick).
  * Output: NO final collective.  Each core writes its full fp32 partial
    [256, B]; the host sums the 8 partials and adds b1.
  * Engine roles: SP DGE streams weights + Y2_x only; Pool DGE issues all
    latency-critical glue DMAs, indirect gathers and collectives; ACT does
    PSUM evacuation + bias; DVE does feature generation only.
"""

import sys

sys.path.insert(0, "/opt/trn_rl_repo")

import numpy as np

N_CORES = 8
D = 512          # d_in == hidden
B = 512          # batch
H = 512          # hidden
DOUT = 256
ROT = D // N_CORES          # 64
KT = 130                    # k-tiles per core
QUAD_BASE = D
CUBIC_BASE = D + (D * D + D) // 2    # 131840
N_H_TILES = H // 128        # 4
N_O_TILES = DOUT // 128     # 2
WCHUNK = 4                  # k-tiles per weight DMA
LAG = 80                    # half-B k-tile lag behind half-A
BH = B // 2                 # 256 batch cols per half

# ---------------------------------------------------------------------------
# Tile schedule: position s -> logical tile.
#  ('SQ',): rows 0-63 squares x_i^2, rows 64-127 cubes x_i^3   (i=64c+p)
#  ('D256',): rows 0-31 pairs (a, a+256) a=32c+k, rows 32-127 zero pad
#  ('L255',): rows 0-63 linear x_i (i=64c+p), rows 64-127 class d=255
#  ('Q', t): rows 0-63 class d1=2t+2, rows 64-127 class d0=2t+1
# Quad tiles ordered by ascending Y2 column m = 31 - t%32 so the layer-1 shift
# array can be consumed as it is built.
# ---------------------------------------------------------------------------
PROC = [("SQ",), ("D256",), ("L255",)]
for _col in range(32):
    for _t in (31 - _col, 63 - _col, 95 - _col, 127 - _col):
        if _t <= 126:
            PROC.append(("Q", _t))
assert len(PROC) == KT


def _triu_idx(lo, hi):
    return QUAD_BASE + lo * D - lo * (lo - 1) // 2 + (hi - lo)


def _pair_fk(i, d):
    """Feature index for pair {i, (i+d) mod D} (arrays ok)."""
    j = (i + d) % D
    lo = np.minimum(i, j)
    hi = np.maximum(i, j)
    return _triu_idx(lo, hi)


def _core_refk(c):
    """W-row (feature) index for each tile row, in PROC order. -1 = zero pad."""
    refk = np.full((KT, 128), -1, dtype=np.int64)
    p = np.arange(64)
    base = ROT * c
    for s, desc in enumerate(PROC):
        if desc[0] == "SQ":
            i = (base + p) % D
            refk[s, 0:64] = _triu_idx(i, i)
            refk[s, 64:128] = CUBIC_BASE + i
        elif desc[0] == "D256":
            a = 32 * c + np.arange(32)
            refk[s, 0:32] = _triu_idx(a, a + 256)
        elif desc[0] == "L255":
            i = (base + p) % D
            refk[s, 0:64] = i
            refk[s, 64:128] = _pair_fk((base + 1 + p) % D, 255)
        else:
            t = desc[1]
            d1, d0 = 2 * t + 2, 2 * t + 1
            r1 = (-d1) % 64
            r0 = r1 + 1
            refk[s, 0:64] = _pair_fk((base + r1 + p) % D, d1)
            refk[s, 64:128] = _pair_fk((base + r0 + p) % D, d0)
    return refk


def _pack_y2(rows16T, b):
    """Parity-split shift array: Y2[64*(r%2)+p, r//2, :] = src[r + p]."""
    y2 = np.zeros((128, 32, b), dtype=np.float16)
    for r in range(64):
        y2[64 * (r % 2):64 * (r % 2) + 64, r // 2, :] = rows16T[r:r + 64]
    return y2


def _prep_core_inputs(c, x16T, W0T, W1T, b0):
    refk = _core_refk(c)

    w0t = np.zeros((KT, 128, H), dtype=np.float16)
    w1t = np.zeros((KT, 128, DOUT), dtype=np.float16)
    m = refk >= 0
    w0t[m] = W0T[refk[m]]
    w1t[m] = W1T[refk[m]]

    frame = x16T[(ROT * c + np.arange(320)) % D]      # rotated 320-row frame
    y2x = _pack_y2(frame, B)

    p = np.arange(64)
    xrep = np.zeros((128, 5, B), dtype=np.float16)
    for m5 in range(5):
        xrep[0:64, m5] = frame[64 * m5 + p]
        xrep[64:128, m5] = frame[64 * m5 + p]
    xmix = np.zeros((128, 2, B), dtype=np.float16)
    xmix[0:64, :, :] = 1.0
    xmix[64:128, 0] = frame[p]
    xmix[64:128, 1] = frame[256 + p]
    xd256 = np.zeros((128, 2, B), dtype=np.float16)
    k32 = np.arange(32)
    xd256[0:32, 0] = x16T[(32 * c + k32) % D]
    xd256[0:32, 1] = x16T[(32 * c + 256 + k32) % D]

    # indirect-gather row indices for the layer-1 h-frame rebuild
    pp = np.arange(128)
    hfridx = np.zeros((128, 4), dtype=np.int32)
    for q in range(3):
        hfridx[:, q] = (ROT * c + 128 * q + pp) % D
    hfridx[0:32, 3] = (32 * c + k32) % D
    hfridx[32:64, 3] = (32 * c + 256 + k32) % D

    b0p = b0[ROT * c:ROT * c + 64].astype(np.float32).reshape(64, 1)

    return {
        "w0t": w0t,
        "w1t": w1t,
        "y2x": y2x,
        "xrep": xrep,
        "xmix": xmix,
        "xd256": xd256,
        "hfridx": hfridx,
        "b0p": b0p,
    }


def _emit_gen(nc, s, ft, y2, xrep, xmix, xd256, sq, lo, hi):
    """One fused gen op for tile s into ft (batch-half cols [lo, hi))."""
    desc = PROC[s]
    v = nc.vector
    if desc[0] == "SQ":
        v.tensor_mul(sq[:, lo:hi], xrep[:, 0, lo:hi], xrep[:, 0, lo:hi])
        v.tensor_mul(ft[:, :], sq[:, lo:hi], xmix[:, 0, lo:hi])
    elif desc[0] == "D256":
        v.tensor_mul(ft[:, :], xd256[:, 0, lo:hi], xd256[:, 1, lo:hi])
    elif desc[0] == "L255":
        v.tensor_mul(ft[:, :], y2[:, 0, lo:hi], xmix[:, 1, lo:hi])
    else:
        t = desc[1]
        v.tensor_mul(ft[:, :], y2[:, 31 - (t % 32), lo:hi],
                     xrep[:, t // 32 + 1, lo:hi])


def _build_program():
    import concourse.mybir as mybir
    import concourse.tile as tile
    from concourse import bacc
    from concourse.bass import AP, IndirectOffsetOnAxis

    fp16 = mybir.dt.float16
    f32 = mybir.dt.float32
    i32 = mybir.dt.int32
    Copy = mybir.ActivationFunctionType.Copy
    Ident = mybir.ActivationFunctionType.Identity
    CORE_IDS = list(range(N_CORES))

    nc = bacc.Bacc(None, target_bir_lowering=False, debug=False)
    with tile.TileContext(nc) as tc:
        with tc.tile_pool(name="dram", bufs=1, space="DRAM") as dram, \
             tc.tile_pool(name="const", bufs=1) as const, \
             tc.tile_pool(name="w0pool", bufs=LAG // WCHUNK + 2) as w0pool, \
             tc.tile_pool(name="w1pool", bufs=6) as w1pool, \
             tc.tile_pool(name="fpool", bufs=8) as fpool, \
             tc.tile_pool(name="spool", bufs=2) as spool, \
             tc.tile_pool(name="hpool", bufs=2) as hpool, \
             tc.tile_pool(name="ps", bufs=1, space="PSUM") as ps:
            # ---- DRAM I/O ----
            w0t = dram.tile([KT, 128, H], fp16, kind="ExternalInput", name="w0t", uniquify=False)
            w1t = dram.tile([KT, 128, DOUT], fp16, kind="ExternalInput", name="w1t", uniquify=False)
            y2xd = dram.tile([128, 32, B], fp16, kind="ExternalInput", name="y2x", uniquify=False)
            xrepd = dram.tile([128, 5, B], fp16, kind="ExternalInput", name="xrep", uniquify=False)
            xmixd = dram.tile([128, 2, B], fp16, kind="ExternalInput", name="xmix", uniquify=False)
            xd256d = dram.tile([128, 2, B], fp16, kind="ExternalInput", name="xd256", uniquify=False)
            hfridxd = dram.tile([128, 4], i32, kind="ExternalInput", name="hfridx", uniquify=False)
            b0pd = dram.tile([64, 1], f32, kind="ExternalInput", name="b0p", uniquify=False)
            outp = dram.tile([DOUT, B], f32, kind="ExternalOutput", name="outp", uniquify=False)

            halves = []
            for hn, lo in (("A", 0), ("B", BH)):
                halves.append({
                    "lo": lo, "hi": lo + BH, "tag": hn,
                    "cc": dram.tile([H, BH], fp16, name=f"cc{hn}", uniquify=False),
                    "rs": dram.tile([ROT, BH], fp16, name=f"rs{hn}", uniquify=False),
                    "hpc": dram.tile([ROT, BH], fp16, name=f"hpc{hn}", uniquify=False),
                    "ag": dram.tile([H, BH], fp16, name=f"ag{hn}", uniquify=False,
                                    addr_space="Shared"),
                    "hfr": dram.tile([448, BH], fp16, name=f"hfr{hn}", uniquify=False),
                })

            # ---- small constants (Pool DGE: fast issue) ----
            xrep_sb = const.tile([128, 5, B], fp16)
            nc.gpsimd.dma_start(out=xrep_sb, in_=xrepd[:])
            xmix_sb = const.tile([128, 2, B], fp16)
            nc.gpsimd.dma_start(out=xmix_sb, in_=xmixd[:])
            xd256_sb = const.tile([128, 2, B], fp16)
            nc.gpsimd.dma_start(out=xd256_sb, in_=xd256d[:])
            hfridx_sb = const.tile([128, 4], i32)
            nc.gpsimd.dma_start(out=hfridx_sb, in_=hfridxd[:])
            b0p_sb = const.tile([64, 1], f32)
            nc.gpsimd.dma_start(out=b0p_sb, in_=b0pd[:])

            # Y2_x in 4 chunks on ACT DGE (keeps SP free to stream weights)
            y2x_sb = const.tile([128, 32, B], fp16, tag="y2x")
            for (c0, c1) in [(0, 4), (4, 12), (12, 22), (22, 32)]:
                nc.scalar.dma_start(out=y2x_sb[:, c0:c1, :], in_=y2xd[:, c0:c1, :])
            sqA = const.tile([128, B], fp16, tag="sq")

            for hf in halves:
                hf["ps0"] = [ps.tile([128, BH], f32, tag=f"p{hf['tag']}{h}",
                                     name=f"ps0{hf['tag']}{h}")
                             for h in range(N_H_TILES)]

            # ---- weight streaming ----
            def load_w(dram_t, sbs, pool, s0, nmt, tagn, eng):
                nw = min(WCHUNK, KT - s0)
                w_sb = pool.tile([128, WCHUNK, nmt * 128], fp16, tag=tagn)
                eng.dma_start(out=w_sb[:, 0:nw, :],
                              in_=dram_t[s0:s0 + nw].rearrange("k p h -> p k h"))
                sbs[s0] = w_sb

            w0_sbs = {}
            for s0 in range(0, KT, WCHUNK):
                load_w(w0t, w0_sbs, w0pool, s0, N_H_TILES, "w0", nc.sync)

            def emit_l0(hf, s):
                ft = fpool.tile([128, BH], fp16, tag="ft")
                _emit_gen(nc, s, ft, y2x_sb, xrep_sb, xmix_sb, xd256_sb,
                          sqA, hf["lo"], hf["hi"])
                w_sb = w0_sbs[(s // WCHUNK) * WCHUNK]
                kk = s % WCHUNK
                for h in range(N_H_TILES):
                    nc.tensor.matmul(hf["ps0"][h],
                                     w_sb[:, kk, 128 * h:128 * h + 128], ft,
                                     start=(s == 0), stop=(s == KT - 1))

            def emit_evac(hf):
                """PSUM evac + partial-sum DRAM write (ACT queue only)."""
                stage = spool.tile([128, N_H_TILES, BH], fp16, tag="evac")
                for h in range(N_H_TILES):
                    nc.scalar.activation(stage[:, h, :], hf["ps0"][h], Copy)
                    nc.scalar.dma_start(out=hf["cc"][128 * h:128 * h + 128, :],
                                        in_=stage[:, h, :])

            def emit_head(hf):
                """RS -> bias fp16 -> AG.  Collectives on Pool, glue on ACT."""
                nc.gpsimd.collective_compute(
                    "ReduceScatter", mybir.AluOpType.add,
                    replica_groups=[CORE_IDS], ins=[hf["cc"][:]], outs=[hf["rs"][:]],
                )
                rsp = spool.tile([64, BH], fp16, tag="rsp")
                nc.scalar.dma_start(out=rsp, in_=hf["rs"][:])
                rsp16 = spool.tile([64, BH], fp16, tag="rsp16")
                nc.scalar.activation(rsp16, rsp, Ident, bias=b0p_sb[:, 0:1])
                nc.scalar.dma_start(out=hf["hpc"][:], in_=rsp16)
                nc.gpsimd.collective_compute(
                    "AllGather", mybir.AluOpType.bypass,
                    replica_groups=[CORE_IDS], ins=[hf["hpc"][:]], outs=[hf["ag"][:]],
                )

            def emit_tail(hf):
                """Rebuild rotated layer-1 operands from the gathered h."""
                # 4 indirect gathers (Pool-only op) -> SBUF -> DRAM frame copy
                hfs = hpool.tile([128, 4, BH], fp16, tag="hfs")
                for q in range(3):
                    nc.gpsimd.indirect_dma_start(
                        out=hfs[:, q, :], out_offset=None, in_=hf["ag"][:],
                        in_offset=IndirectOffsetOnAxis(ap=hfridx_sb[:, q:q + 1], axis=0),
                    )
                nc.gpsimd.indirect_dma_start(
                    out=hfs[0:64, 3, :], out_offset=None, in_=hf["ag"][:],
                    in_offset=IndirectOffsetOnAxis(ap=hfridx_sb[0:64, 3:4], axis=0),
                )
                hfrd = hf["hfr"]
                for q in range(3):
                    nc.scalar.dma_start(out=hfrd[128 * q:128 * q + 128, :],
                                        in_=hfs[:, q, :])
                nc.scalar.dma_start(out=hfrd[384:448, :], in_=hfs[0:64, 3, :])

                def win(row0, pn, sn, sstride_rows):
                    base = hfrd[row0:row0 + 1, :]
                    return AP(base.tensor, base.offset,
                              [[BH, pn], [sstride_rows * BH, sn], [1, BH]])

                # hrep/hmix/hd256 first (they gate tiles 0-2), then y2h chunks
                hrep = hpool.tile([128, 5, BH], fp16, tag="hrep")
                nc.scalar.dma_start(out=hrep[0:64, :, :], in_=win(0, 64, 5, 64))
                nc.scalar.dma_start(out=hrep[64:128, :, :], in_=win(0, 64, 5, 64))
                hmix = hpool.tile([128, 2, BH], fp16, tag="hmix")
                nc.vector.memset(hmix[0:64, :, :], 1.0)
                nc.scalar.dma_start(out=hmix[64:128, :, :], in_=win(0, 64, 2, 256))
                hd256 = hpool.tile([128, 2, BH], fp16, tag="hd256")
                nc.vector.memset(hd256[32:64, :, :], 0.0)
                nc.vector.memset(hd256[64:128, :, :], 0.0)
                nc.scalar.dma_start(out=hd256[0:32, :, :], in_=win(384, 32, 2, 32))
                y2h = hpool.tile([128, 32, BH], fp16, tag="y2h")
                for (s0, s1) in [(0, 8), (8, 32)]:
                    nc.scalar.dma_start(out=y2h[0:64, s0:s1, :],
                                        in_=win(2 * s0, 64, s1 - s0, 2))
                    nc.scalar.dma_start(out=y2h[64:128, s0:s1, :],
                                        in_=win(2 * s0 + 1, 64, s1 - s0, 2))
                sqh = hpool.tile([128, BH], fp16, tag="sqh")
                hf["h_ops"] = (y2h, hrep, hmix, hd256, sqh)
                hf["ps1"] = [ps.tile([128, BH], f32, tag=f"p{hf['tag']}{h}",
                                     name=f"ps1{hf['tag']}{h}")
                             for h in range(N_O_TILES)]

            def emit_l1(hf, s, w_sbs):
                y2h, hrep, hmix, hd256, sqh = hf["h_ops"]
                ft = fpool.tile([128, BH], fp16, tag="ft")
                _emit_gen(nc, s, ft, y2h, hrep, hmix, hd256, sqh, 0, BH)
                w_sb = w_sbs[(s // WCHUNK) * WCHUNK]
                kk = s % WCHUNK
                for h in range(N_O_TILES):
                    nc.tensor.matmul(hf["ps1"][h],
                                     w_sb[:, kk, 128 * h:128 * h + 128], ft,
                                     start=(s == 0), stop=(s == KT - 1))

            def emit_out(hf):
                lo, hi = hf["lo"], hf["hi"]
                for h in range(N_O_TILES):
                    so = spool.tile([128, BH], f32, tag="oevac")
                    nc.scalar.activation(so, hf["ps1"][h], Copy)
                    nc.gpsimd.dma_start(out=outp[128 * h:128 * h + 128, lo:hi],
                                        in_=so)

            # ================= layer 0: interleaved halves =================
            A, Bhf = halves
            for s in range(KT + LAG):
                if s < KT:
                    emit_l0(A, s)
                if s == KT - 1:
                    emit_evac(A)
                    emit_head(A)
                if s >= LAG:
                    emit_l0(Bhf, s - LAG)
            emit_evac(Bhf)
            emit_tail(A)
            emit_head(Bhf)
            emit_tail(Bhf)

            # ================= layer 1: sequential halves =================
            w1a_sbs = {}
            w1b_sbs = {}
            for s0 in range(0, KT, WCHUNK):
                load_w(w1t, w1a_sbs, w1pool, s0, N_O_TILES, "w1", nc.sync)
            for s0 in range(0, KT, WCHUNK):
                load_w(w1t, w1b_sbs, w1pool, s0, N_O_TILES, "w1", nc.sync)
            for s in range(KT):
                emit_l1(A, s, w1a_sbs)
            emit_out(A)
            for s in range(KT):
                emit_l1(Bhf, s, w1b_sbs)
            emit_out(Bhf)
    nc.compile()
    return nc


_NC_CACHE = None


def build_in_maps(x, W0, b0, W1, b1):
    x16T = np.ascontiguousarray(x.T).astype(np.float16)          # [D, B]
    W0T = np.ascontiguousarray(W0.T).astype(np.float16)          # [K, H]
    W1T = np.ascontiguousarray(W1.T).astype(np.float16)          # [K, DOUT]
    return [_prep_core_inputs(c, x16T, W0T, W1T, b0) for c in range(N_CORES)]


def kernel(x, W0, b0, W1, b1):
    global _NC_CACHE
    from concourse.bass_utils import run_bass_kernel_spmd

    in_maps = build_in_maps(x, W0, b0, W1, b1)
    if _NC_CACHE is None:
        _NC_CACHE = _build_program()
    res = run_bass_kernel_spmd(_NC_CACHE, in_maps, list(range(N_CORES)))
    acc = np.zeros((DOUT, B), dtype=np.float32)
    for c in range(N_CORES):
        acc += res.results[c]["outp"]
    acc += b1.astype(np.float32).reshape(DOUT, 1)
    return np.ascontiguousarray(acc.T)
